# revision 47
# baseline (speedup 1.0000x reference)
"""Multi-head attention (B=4, T=2048, C=1024, H=16, D=64) on 8 TRN2 cores.

Sharding: core i handles batch b=i//2 and the 8 heads of half hh=i%2.
Each core computes its heads' contribution through the row-sharded output
projection -> partial y [T, C]; host sums the two partials per batch.

v2: bf16 compute (FWL weight loads, no fp32 power throttle), software
pipelining to keep the PE HAM-warm:
  P1: dense q/k/v projections for head-group 0 (pairs 0,1)
  P2: attention pairs 0,1 with head-group 1's projection matmuls
      interleaved as PE filler between score steps (exp on ScalarE is
      the attention bottleneck; filler keeps the PE from micro-idling)
  P3: attention pairs 2,3 (normalize of earlier pairs interleaved)
  tail: last normalize + output projection
Causal mask applied post-exp on GpSimd (affine_select fill=0), score
matmuls N-trimmed on diagonal blocks, softmax normalizer via
reciprocal_approx_fast + K=2 selector broadcast matmul.

Per-core layouts (host pre-arranged, bf16):
  xT  [C, T]    = x[b].T
  wq/wk/wv [C, 512]  columns = (local head)*64 + d
  wpt [512, C]  rows  = (local head)*64 + d   (= Wp.T row-slice)
  bp  [C] f32   bias on even cores, zeros on odd (summed partials)
"""

import os
import sys

import numpy as np

for _p in ("/opt/trn_rl_repo", "/root/.axon_site/_ro/trn_rl_repo"):
    if os.path.isdir(_p) and _p not in sys.path:
        sys.path.append(_p)

import ml_dtypes

import concourse.bass as bass
import concourse.bacc as bacc
import concourse.mybir as mybir
import concourse.tile as tile
from concourse.bass_utils import run_bass_kernel_spmd

B, T, C, H, D = 4, 2048, 1024, 16, 64
HL = H // 2          # heads per core
P = 128
NCH = C // P         # 8 c-chunks
NTT = T // 512       # 4 t-tiles of 512
NSB = T // P         # 16 s-blocks of 128
SCALE = 1.0 / 32.0   # 1/sqrt(C)

F32 = mybir.dt.float32
F32R = mybir.dt.float32r
BF16 = mybir.dt.bfloat16
F8 = mybir.dt.float8e4
NPBF16 = ml_dtypes.bfloat16
NPF8 = ml_dtypes.float8_e4m3
W8SCALE = 64.0                     # q/k weights pre-scaled into fp8 range
SC_EXP8 = SCALE / (W8SCALE * W8SCALE)   # exp scale when q,k carry 64x


def _build(causal: bool) -> bass.Bass:
    nc = bacc.Bacc("TRN2", target_bir_lowering=False, debug=False, num_devices=8)

    xT = nc.dram_tensor("xT", [C, T], BF16, kind="ExternalInput").ap()
    xT8_d = nc.dram_tensor("xT8", [C, T], F8, kind="ExternalInput").ap()
    wq_d = nc.dram_tensor("wq", [C, HL * D], F8, kind="ExternalInput").ap()
    wk_d = nc.dram_tensor("wk", [C, HL * D], F8, kind="ExternalInput").ap()
    wv_d = nc.dram_tensor("wv", [C, HL * D], BF16, kind="ExternalInput").ap()
    wpt_d = nc.dram_tensor("wpt", [HL * D, C], BF16, kind="ExternalInput").ap()
    bp_d = nc.dram_tensor("bp", [C], F32, kind="ExternalInput").ap()
    sel_d = nc.dram_tensor("sel", [64, P], BF16, kind="ExternalInput").ap()
    y_d = nc.dram_tensor("y", [T, C], F32, kind="ExternalOutput").ap()

    with tile.TileContext(nc) as tc:
        _emit(nc, tc, causal, xT, xT8_d, wq_d, wk_d, wv_d, wpt_d, bp_d,
              sel_d, y_d)
    nc.compile()
    return nc


def _emit(nc, tc, causal, xT, xT8_d, wq_d, wk_d, wv_d, wpt_d, bp_d, sel_d,
          y_d):
    from collections import deque
    from contextlib import ExitStack

    ctx = ExitStack()
    with ctx:
        consts = ctx.enter_context(tc.tile_pool(name="consts", bufs=1))
        x_pool = ctx.enter_context(tc.tile_pool(name="xh", bufs=1))
        x8_pool = ctx.enter_context(tc.tile_pool(name="xh8", bufs=1))
        wq_pool = ctx.enter_context(tc.tile_pool(name="wq", bufs=2))
        wk_pool = ctx.enter_context(tc.tile_pool(name="wk", bufs=2))
        wv_pool = ctx.enter_context(tc.tile_pool(name="wv", bufs=2))
        q_pool = ctx.enter_context(tc.tile_pool(name="qT", bufs=4))
        k_pool = ctx.enter_context(tc.tile_pool(name="kT", bufs=4))
        v_pool = ctx.enter_context(tc.tile_pool(name="v", bufs=2))
        oc_pool = ctx.enter_context(tc.tile_pool(name="outcat", bufs=4))
        p_pool = ctx.enter_context(tc.tile_pool(name="pT", bufs=5))
        z_pool = ctx.enter_context(tc.tile_pool(name="zb", bufs=3))
        rz_pool = ctx.enter_context(tc.tile_pool(name="rz", bufs=2))
        rzs_pool = ctx.enter_context(tc.tile_pool(name="rzs", bufs=1))
        rzs3_pool = ctx.enter_context(tc.tile_pool(name="rzs3", bufs=2))
        yst_pool = ctx.enter_context(tc.tile_pool(name="yst", bufs=3))
        wpt_pool = ctx.enter_context(tc.tile_pool(name="wpt", bufs=4))
        bpb_pool = ctx.enter_context(tc.tile_pool(name="bpb", bufs=1))
        psA = ctx.enter_context(tc.tile_pool(name="psA", bufs=2, space="PSUM"))
        pso = ctx.enter_context(tc.tile_pool(name="pso", bufs=2, space="PSUM"))
        # psP lives through P1/P2 only; its 2 banks are then recycled for
        # pair 3's outp tiles during the dual phase, then for the tail
        psP = tc.alloc_tile_pool(name="psP", bufs=2, space="PSUM")

        # Normalizer broadcast selector (host constant): row 0 -> out rows
        # 0-63 (u=0), row 32 -> out rows 64-127 (u=1); all other rows zero
        # so garbage in the unused rz partitions is multiplied by 0.
        sel = consts.tile([64, P], BF16)

        # ---- input DMAs; two HWDGE queues in parallel: sync carries the
        # fp8 q/k path (needed first), scalar carries the bf16 x / wv /
        # wpt / bpb path. Ordered so the first projection group and the
        # first PVs are ready ASAP. ----
        xh = x_pool.tile([P, NCH, T], BF16, tag="xh")
        xh8 = x8_pool.tile([P, NCH, T], F8, tag="xh8")
        wts = {}
        for hg in range(2):
            wq_t = wq_pool.tile([P, NCH, 4 * D], F8, tag="wq",
                                name=f"wq{hg}")
            wk_t = wk_pool.tile([P, NCH, 4 * D], F8, tag="wk",
                                name=f"wk{hg}")
            wv_t = wv_pool.tile([P, NCH, 4 * D], BF16, tag="wv",
                                name=f"wv{hg}")
            wts[hg] = (wq_t, wk_t, wv_t)

        def dma_w(hg, idx, eng):
            hsl = slice(hg * 4 * D, (hg + 1) * 4 * D)
            w_d = (wq_d, wk_d, wv_d)[idx]
            eng.dma_start(
                out=wts[hg][idx],
                in_=w_d[:, hsl].rearrange("(n p) d -> p n d", p=P))

        def dma_x(w, dst, src, eng):
            for c in range(NCH):
                eng.dma_start(
                    out=dst[:, c, w * 512:(w + 1) * 512],
                    in_=src[c * P:(c + 1) * P, w * 512:(w + 1) * 512])

        # Two HWDGE queues in parallel. Scalar carries only a handful of
        # descriptors (cheap on the ACT stream) for the first-needed
        # weights; sync carries the bulk x transfers in consumption order.
        dma_w(0, 0, nc.scalar)
        dma_w(0, 1, nc.scalar)
        dma_x(0, xh8, xT8_d, nc.sync)
        dma_x(1, xh8, xT8_d, nc.sync)
        dma_x(0, xh, xT, nc.scalar)
        dma_w(0, 2, nc.scalar)
        dma_x(2, xh8, xT8_d, nc.sync)
        dma_x(3, xh8, xT8_d, nc.sync)
        dma_x(1, xh, xT, nc.scalar)
        dma_x(2, xh, xT, nc.sync)
        dma_x(3, xh, xT, nc.sync)
        dma_w(1, 0, nc.scalar)
        dma_w(1, 1, nc.scalar)
        dma_w(1, 2, nc.scalar)
        nc.sync.dma_start(out=sel, in_=sel_d)
        # gpsimd software DGE: late-needed output-projection constants
        wpt_t = [wpt_pool.tile([P, C], BF16, tag="wpt", name=f"wpt{i}")
                 for i in range(4)]
        for q in range(4):
            nc.gpsimd.dma_start(out=wpt_t[q], in_=wpt_d[q * P:(q + 1) * P, :])
        bpb = bpb_pool.tile([P, C], F32)
        nc.gpsimd.dma_start(
            out=bpb,
            in_=bass.AP(tensor=bp_d.tensor, offset=0, ap=[[0, P], [1, C]]),
        )

        qT2 = [None] * 4   # per pair [128 (2 heads x 64d), T] bf16
        kT2 = [None] * 4
        v_t = [None] * 2   # per head-group [128 s, NSB, 4, D+1] bf16
        outcat = [oc_pool.tile([P, T], BF16, tag="outcat", name=f"outcat{i}")
                  for i in range(4)]
        zb = [None] * 4    # per pair [128, NTT, 512] f32; rows 0(u0)/32(u1)
        rz = [None] * 4

        # ---------------- projection pieces (filler-granular) ----------
        def alloc_proj(hg):
            for pr in range(2):
                pair = hg * 2 + pr
                qT2[pair] = q_pool.tile([P, T], BF16, tag="qT",
                                        name=f"qT{pair}")
                kT2[pair] = k_pool.tile([P, T], BF16, tag="kT",
                                        name=f"kT{pair}")
            v_t[hg] = v_pool.tile([P, NSB, 4, D + 1], BF16, tag="v",
                                  name=f"v{hg}")
            nc.vector.memset(v_t[hg][:, :, :, D:], 1.0)

        def qk_group(hg, th, pr, which, tt, on_act):
            # fp8 DoubleRow: each matmul contracts a pair of c-chunks
            # ([128, 2, .] APs); q/k tiles carry a 64x weight scale that
            # the exp scale divides back out.
            w_t = wts[hg][which]
            dst = (qT2 if which == 0 else kT2)[hg * 2 + pr]
            t0 = th * 1024 + tt * 512
            ps = psP.tile([P, 512], F32, tag="psP", name="qk")
            for g in range(NCH // 2):
                nc.tensor.matmul(
                    ps, w_t[:, 2 * g:2 * g + 2, pr * P:(pr + 1) * P],
                    xh8[:, 2 * g:2 * g + 2, t0:t0 + 512],
                    start=g == 0, stop=g == NCH // 2 - 1,
                    perf_mode=mybir.MatmulPerfMode.DoubleRow)
            if on_act:
                nc.scalar.copy(out=dst[:, t0:t0 + 512], in_=ps)
            else:
                nc.vector.tensor_copy(out=dst[:, t0:t0 + 512], in_=ps)

        def v_group(hg, g):   # g in 0..7 covers s [g*256, (g+1)*256)
            th, sbp = g // 4, g % 4
            wv_t = wts[hg][2]
            vps = psP.tile([P, 2, 256], F32, tag="psP", name="vps")
            for c in range(NCH):
                for u in range(2):
                    s0 = th * 1024 + (sbp * 2 + u) * P
                    nc.tensor.matmul(
                        vps[:, u, :],
                        xh[:, c, s0:s0 + P],
                        wts[hg][2][:, c, :],
                        start=(c == 0 and u == 0), stop=c == NCH - 1)
            sb0 = th * 8 + sbp * 2
            nc.vector.tensor_copy(
                out=v_t[hg][:, sb0:sb0 + 2, :, 0:D],
                in_=vps.rearrange("p u (h d) -> p u h d", h=4))

        # ---------------- normalize ----------------
        def norm_recip(pair):
            rz[pair] = rz_pool.tile([P, NTT, 512], BF16, tag="rz",
                                    name=f"rz{pair}")
            scratch = rzs_pool.tile([64, NTT, 512], F32, tag="rzs")
            nc.vector.reciprocal_approx_fast(
                out=scratch, in_=zb[pair][0:64, :, :])
            nc.vector.tensor_copy(out=rz[pair][0:64, :, :], in_=scratch)

        def norm_recip_j(pair, j):
            scratch = rzs3_pool.tile([64, 512], F32, tag="rzs3")
            nc.vector.reciprocal_approx_fast(
                out=scratch, in_=zb[pair][0:64, j, :])
            nc.vector.tensor_copy(out=rz[pair][0:64, j, :], in_=scratch)

        def norm_j(pair, j, pspool):
            bps = pspool.tile([P, 512], F32, tag="psP", name="bps")
            nc.tensor.matmul(bps, sel, rz[pair][0:64, j, :],
                             start=True, stop=True)
            osl = outcat[pair][:, j * 512:(j + 1) * 512]
            nc.vector.tensor_mul(osl, osl, bps)

        def emit_normalize(pair, pspool):
            norm_recip(pair)
            for j in range(NTT):
                norm_j(pair, j, pspool)

        ydma_flip = [0]

        def yproj_group(m, n, pspool):
            yps = pspool.tile([P, 512], F32, tag="psP", name="yps")
            for q in range(4):
                nc.tensor.matmul(
                    yps,
                    outcat[q][:, m * P:(m + 1) * P],
                    wpt_t[q][:, n * 512:(n + 1) * 512],
                    start=(q == 0), stop=(q == 3))
            yt = yst_pool.tile([P, 512], F32, tag="yst", name="yt")
            nc.vector.tensor_add(yt, yps, bpb[:, n * 512:(n + 1) * 512])
            eng = nc.sync if ydma_flip[0] % 2 == 0 else nc.scalar
            ydma_flip[0] += 1
            eng.dma_start(
                out=y_d[m * P:(m + 1) * P, n * 512:(n + 1) * 512],
                in_=yt)

        # ---------------- attention ----------------
        def alloc_zb(pair):
            zb[pair] = z_pool.tile([P, NTT, 512], F32, tag="zb",
                                   name=f"zb{pair}")
            # rows 1-31/33-63 are never written but feed the (zero-weighted)
            # reciprocal input; keep them finite
            nc.vector.memset(zb[pair][0:64, :, :], 1.0)

        def attention_pair(pair, fq, fill_every, on_j_done=None):
            hg, pr = pair // 2, pair % 2
            step = 0
            for j in range(NTT):
                nsb_j = 4 * (j + 1) if causal else NSB
                outp = [pso.tile([D + 1, 512], F32, tag="pso",
                                 name=f"outp{i}") for i in range(2)]

                def emit_pv(i, lo, last):
                    for u in range(2):
                        nc.tensor.matmul(
                            outp[u][:, lo:512],
                            v_t[hg][:, i, pr * 2 + u, :],
                            pend[i][:, u, lo:512],
                            start=(i == 0), stop=last,
                            skip_group_check=True)
                    del pend[i]

                pend = {}
                prev = None
                for i in range(nsb_j):
                    r = i - 4 * j if causal else -1
                    lo = max(r, 0) * P
                    last = i == nsb_j - 1
                    scs = psA.tile([P, 2, 512], F32, tag="psA", name="scs")
                    pts = p_pool.tile([P, 2, 512], BF16, tag="pT", name="pts")
                    pend[i] = pts
                    for u in range(2):
                        dsl = slice(u * D, (u + 1) * D)
                        nc.tensor.matmul(
                            scs[:, u, lo:512],
                            kT2[pair][dsl, i * P:(i + 1) * P],
                            qT2[pair][dsl, j * 512 + lo:(j + 1) * 512],
                            start=True, stop=True)
                    nc.scalar.activation(
                        out=pts[:, :, lo:512],
                        in_=scs[:, :, lo:512],
                        func=mybir.ActivationFunctionType.Exp,
                        scale=SC_EXP8)
                    if causal and r >= 0:
                        # zero the upper triangle of the diagonal block
                        # post-exp (GpSimd; keeps DVE/ScalarE free)
                        nc.gpsimd.affine_select(
                            out=pts[:, :, lo:lo + P],
                            in_=pts[:, :, lo:lo + P],
                            compare_op=mybir.AluOpType.is_ge,
                            fill=0.0, base=0,
                            pattern=[[0, 2], [1, P]], channel_multiplier=-1,
                        )
                    if prev is not None:
                        emit_pv(*prev)
                    prev = (i, lo, last)
                    step += 1
                    if fq and fill_every and step % fill_every == 0:
                        fq.popleft()()
                if prev is not None:
                    emit_pv(*prev)
                for u in range(2):
                    nc.vector.tensor_copy(
                        out=outcat[pair][u * D:(u + 1) * D,
                                         j * 512:(j + 1) * 512],
                        in_=outp[u][0:D, :])
                    nc.vector.tensor_copy(
                        out=zb[pair][32 * u:32 * u + 1, j, :],
                        in_=outp[u][D:D + 1, :])
                if on_j_done is not None:
                    on_j_done(j)

        def attention_dual(pA, pB, poolB):
            """Step-interleave two pairs: each pair's exp latency hides
            under the other pair's PE work; ScalarE runs saturated."""
            st = {p: {"pend": {}, "prev": None, "outp": None}
                  for p in (pA, pB)}

            def emit_pv(pair, i, lo, last):
                s = st[pair]
                hg, pr = pair // 2, pair % 2
                for u in range(2):
                    nc.tensor.matmul(
                        s["outp"][u][:, lo:512],
                        v_t[hg][:, i, pr * 2 + u, :],
                        s["pend"][i][:, u, lo:512],
                        start=(i == 0), stop=last,
                        skip_group_check=True)
                del s["pend"][i]

            for j in range(NTT):
                nsb_j = 4 * (j + 1) if causal else NSB
                for pair in (pA, pB):
                    pool = pso if pair == pA else poolB
                    tg = "pso" if pair == pA else "psoB"
                    st[pair]["outp"] = [
                        pool.tile([D + 1, 512], F32, tag=tg,
                                  name=f"outp{pair}_{i}") for i in range(2)]
                for i in range(nsb_j):
                    r = i - 4 * j if causal else -1
                    lo = max(r, 0) * P
                    last = i == nsb_j - 1
                    for pair in (pA, pB):
                        s = st[pair]
                        scs = psA.tile([P, 2, 512], F32, tag="psA",
                                       name="scs")
                        pts = p_pool.tile([P, 2, 512], BF16, tag="pT",
                                          name="pts")
                        s["pend"][i] = pts
                        for u in range(2):
                            dsl = slice(u * D, (u + 1) * D)
                            nc.tensor.matmul(
                                scs[:, u, lo:512],
                                kT2[pair][dsl, i * P:(i + 1) * P],
                                qT2[pair][dsl, j * 512 + lo:(j + 1) * 512],
                                start=True, stop=True)
                        nc.scalar.activation(
                            out=pts[:, :, lo:512],
                            in_=scs[:, :, lo:512],
                            func=mybir.ActivationFunctionType.Exp,
                            scale=SC_EXP8)
                        if causal and r >= 0:
                            nc.gpsimd.affine_select(
                                out=pts[:, :, lo:lo + P],
                                in_=pts[:, :, lo:lo + P],
                                compare_op=mybir.AluOpType.is_ge,
                                fill=0.0, base=0,
                                pattern=[[0, 2], [1, P]],
                                channel_multiplier=-1,
                            )
                        if s["prev"] is not None:
                            emit_pv(pair, *s["prev"])
                        s["prev"] = (i, lo, last)
                for pair in (pA, pB):
                    s = st[pair]
                    if s["prev"] is not None:
                        emit_pv(pair, *s["prev"])
                        s["prev"] = None
                    for u in range(2):
                        nc.vector.tensor_copy(
                            out=outcat[pair][u * D:(u + 1) * D,
                                             j * 512:(j + 1) * 512],
                            in_=s["outp"][u][0:D, :])
                        nc.vector.tensor_copy(
                            out=zb[pair][32 * u:32 * u + 1, j, :],
                            in_=s["outp"][u][D:D + 1, :])
                    # per-j reciprocal on DVE (no PSUM involved) so the
                    # tail's broadcast matmuls start immediately
                    norm_recip_j(pair, j)

        # ================= schedule =================
        from functools import partial

        # P1: head-group 0 q/k projections, dense, ordered by x window so
        # the PE starts as soon as window 0 lands and never gaps (HAM
        # warm-up); copies alternate ACT/DVE. v(hg0) windows 0,1 emitted
        # here so pair 0's first PVs never wait.
        alloc_proj(0)
        flip = 0
        for th in range(2):
            for tt in range(2):
                for pr in range(2):
                    for which in range(2):
                        qk_group(0, th, pr, which, tt, on_act=flip % 2 == 0)
                        flip += 1
        v_group(0, 0)
        v_group(0, 1)

        # P2: attention pairs 0,1; filler = rest of v(hg0), all of
        # head-group 1's projections, and normalize(0)
        alloc_proj(1)
        alloc_zb(0)
        alloc_zb(1)
        fq = deque()
        for g in range(2, 8):
            fq.append(partial(v_group, 0, g))
        for th in range(2):
            for tt in range(2):
                for pr in range(2):
                    for which in range(2):
                        fq.append(partial(qk_group, 1, th, pr, which, tt,
                                          False))
        for g in range(8):
            fq.append(partial(v_group, 1, g))
        attention_pair(0, fq, 3)
        fq.append(partial(emit_normalize, 0, psP))
        attention_pair(1, fq, 2)
        while fq:
            fq.popleft()()
        emit_normalize(1, psP)
        psP.release()

        # P3: pairs 2,3 step-interleaved; pair 3's outp tiles take the
        # released psP banks
        alloc_zb(2)
        alloc_zb(3)
        rz[2] = rz_pool.tile([P, NTT, 512], BF16, tag="rz", name="rz2")
        rz[3] = rz_pool.tile([P, NTT, 512], BF16, tag="rz", name="rz3")
        psoB = tc.alloc_tile_pool(name="psoB", bufs=2, space="PSUM")
        attention_dual(2, 3, psoB)
        psoB.release()

        # tail: normalizer broadcasts + output projection, j-major so the
        # PE stream stays dense
        psPt = tc.alloc_tile_pool(name="psPt", bufs=2, space="PSUM")
        for j in range(NTT):
            norm_j(2, j, psPt)
            norm_j(3, j, psPt)
            for m in range(4 * j, 4 * j + 4):
                for n in range(2):
                    yproj_group(m, n, psPt)
        psPt.release()


_NC_CACHE = {}
LAST_RESULTS = None


def kernel(x, Wq, Wk, Wv, Wp, bp, is_masked, **_unused):
    global LAST_RESULTS
    x = np.asarray(x, np.float32)
    Wq = np.asarray(Wq, np.float32)
    Wk = np.asarray(Wk, np.float32)
    Wv = np.asarray(Wv, np.float32)
    Wp = np.asarray(Wp, np.float32)
    bp = np.asarray(bp, np.float32)
    causal = bool(np.asarray(is_masked).item())

    if causal not in _NC_CACHE:
        _NC_CACHE[causal] = _build(causal)
    nc = _NC_CACHE[causal]

    # host-side layout prep (bf16 for v/output path, fp8 for q/k path)
    wq_r = np.ascontiguousarray(
        Wq.transpose(1, 0, 2).reshape(C, H * D) * W8SCALE).astype(NPF8)
    wk_r = np.ascontiguousarray(
        Wk.transpose(1, 0, 2).reshape(C, H * D) * W8SCALE).astype(NPF8)
    wv_r = np.ascontiguousarray(
        Wv.transpose(1, 0, 2).reshape(C, H * D)).astype(NPBF16)
    wpt = np.ascontiguousarray(Wp.T).astype(NPBF16)
    zeros = np.zeros_like(bp)

    sel = np.zeros((64, P), np.float32)
    sel[0, 0:64] = 1.0
    sel[32, 64:128] = 1.0
    sel = sel.astype(NPBF16)

    xTs = [np.ascontiguousarray(x[b].T).astype(NPBF16) for b in range(B)]
    xTs8 = [np.ascontiguousarray(x[b].T).astype(NPF8) for b in range(B)]
    in_maps = []
    for core in range(8):
        b, hh = core // 2, core % 2
        csl = slice(hh * HL * D, (hh + 1) * HL * D)
        in_maps.append({
            "xT": xTs[b],
            "xT8": xTs8[b],
            "wq": np.ascontiguousarray(wq_r[:, csl]),
            "wk": np.ascontiguousarray(wk_r[:, csl]),
            "wv": np.ascontiguousarray(wv_r[:, csl]),
            "wpt": np.ascontiguousarray(wpt[csl, :]),
            "bp": bp if hh == 0 else zeros,
            "sel": sel,
        })

    trace = bool(int(os.environ.get("KERNEL_TRACE", "0")))
    res = run_bass_kernel_spmd(
        nc, in_maps, core_ids=list(range(8)), trace=trace)
    LAST_RESULTS = res

    y = np.empty((B, T, C), np.float32)
    for b in range(B):
        y[b] = res.results[2 * b]["y"] + res.results[2 * b + 1]["y"]
    return y
